# revision 1
# baseline (speedup 1.0000x reference)
"""Trainium2 Bass kernel for nn_Attention_85005992722686.

Head-sharded tensor-parallel causal attention over 8 NeuronCores.
Core c owns heads {2c, 2c+1}; layernorms are algebraically folded:

  y = softmax(causal((LN(x;g,b) @ Wq) (LN(x;gc,bc) @ Wk)^T / 8)) @ (LN(x) @ Wv) @ Wo

Per core (hd = 128 = 2 heads x 64):
  host:   Wq_eff = g*Wq*0.125, Wk_eff = gc*Wk, Wv_eff = gc*Wv (column shards),
          ncs_* = -colsum(W*_eff), Wo row-shard, xT = x.transpose (layout only)
  device: S1/S2 column stats via PE gram matmuls -> mean, rstd
          P_T = W_eff^T @ xT  (+ rank-1 -colsum x mean via K=1 matmul)
          qT/kT/vT = P_T * rstd_bcast   (DVE eviction fused)
          v_nat = PE-transpose(vT), augmented with ones column
          S^T[j,i] = kT^T qT (causal blocks only), P = exp(S^T), tri-mask diag
          [attn^T; denom] = [v|1]^T @ P^T   (PSUM accumulate over j)
          attnhat^T = attn^T * (1/denom bcast)
          y_partial = attnhat^T^T @ Wo_shard
  host:   y = sum of 8 partial y
"""
import sys
sys.path.insert(0, '/opt/trn_rl_repo')
import numpy as np
import concourse.bass as bass
import concourse.bacc as bacc
import concourse.tile as tile
from concourse import mybir
from concourse.bass_utils import run_bass_kernel_spmd

F32 = mybir.dt.float32
F32R = mybir.dt.float32r
AF = mybir.ActivationFunctionType
ALU = mybir.AluOpType

B, N, D = 2, 2048, 1024
H, DH = 16, 64
EPS = 1e-5
NCORES = 8
HD = 128          # head-dim slice per core (2 heads x 64)
KT = D // 128     # 8 k-tiles over model dim
NT = N // 128     # 16 n-tiles
NCH = N // 512    # 4 n-chunks of 512
BLK = 258         # xT block: 256 data cols + 2 ones cols (fp32r wants even counts)

USE_F32R = True   # False -> plain fp32 matmuls (4x slower, bit-safer)
STAGE = 60        # debug: truncate pipeline (1 loads, 2 stats, 3 proj, 4 vaug, 5 attn, 6 full)
TRACE = False
TRACE_KWARGS = {}
LAST_RESULTS = None


def _mmdt():
    return F32R if USE_F32R else F32


def _build_program(with_bias):
    MDT = _mmdt()
    nc = bacc.Bacc("TRN2", target_bir_lowering=False, debug=False,
                   num_devices=NCORES)
    # ---------------- dram io ----------------
    xt_d = nc.dram_tensor("xt", [B, D, NCH * 2 * BLK], MDT, kind="ExternalInput")
    wqkv_d = nc.dram_tensor("wqkv", [D, 3 * HD], MDT, kind="ExternalInput")
    wo_d = nc.dram_tensor("wo", [HD, D], MDT, kind="ExternalInput")
    # aux row: [ncs_q | ncs_k | ncs_v | ones] each 128 wide
    aux_d = nc.dram_tensor("aux", [1, 512], MDT, kind="ExternalInput")
    tri_d = nc.dram_tensor("tri", [128, 128], MDT, kind="ExternalInput")
    ident_d = nc.dram_tensor("ident", [128, 128], F32, kind="ExternalInput")
    if with_bias:
        bias_d = nc.dram_tensor("biasr", [1, 384], MDT, kind="ExternalInput")
    y_d = nc.dram_tensor("y", [B, N, D], F32, kind="ExternalOutput")

    with tile.TileContext(nc) as tc:
        with tc.tile_pool(name="wpool", bufs=1) as wpool, \
             tc.tile_pool(name="xpool", bufs=1) as xpool, \
             tc.tile_pool(name="big", bufs=1) as bigp, \
             tc.tile_pool(name="small", bufs=1) as smallp, \
             tc.tile_pool(name="pstrip", bufs=6) as ppool, \
             tc.tile_pool(name="psA", bufs=4, space="PSUM") as psA, \
             tc.tile_pool(name="psB", bufs=4, space="PSUM") as psB:

            # ---- very first: b0 chunk-0 x tiles (gate the first grams) ----
            xt_first = {}
            for kt in range(KT):
                t = xpool.tile([128, 2 * BLK], MDT, name=f"xt0_{kt}_0",
                               tag=f"xt{kt}_0")
                nc.sync.dma_start(
                    t[:], xt_d.ap()[0, kt * 128:(kt + 1) * 128, 0:2 * BLK])
                xt_first[kt] = t

            # ---- early statics: weights/ident/aux gate the first chunk ----
            w_sb = {}
            for kt in range(KT):
                t = wpool.tile([128, 3 * HD], MDT, name=f"wqkv{kt}")
                nc.sync.dma_start(t[:], wqkv_d.ap()[kt * 128:(kt + 1) * 128, :])
                for ti, nm in enumerate(("q", "k", "v")):
                    w_sb[nm, kt] = t[:, ti * HD:(ti + 1) * HD]
            ident_sb = wpool.tile([128, 128], F32, name="ident_sb")
            nc.sync.dma_start(ident_sb[:], ident_d.ap()[:, :])
            aux_sb = wpool.tile([1, 512], MDT, name="aux_sb")
            nc.sync.dma_start(aux_sb[:], aux_d.ap()[:, :])
            if with_bias:
                bias_sb = wpool.tile([1, 384], MDT, name="bias_sb")
                nc.sync.dma_start(bias_sb[:], bias_d.ap()[:, :])
            ones_row = aux_sb[0:1, 384:512]        # [1, 128] of ones

            # ---- b=0 remaining xt loads ----
            xt_sb_all = {0: {}}
            for bp in range(NCH):
                for kt in range(KT):
                    if bp == 0:
                        xt_sb_all[0][kt, 0] = xt_first[kt]
                        continue
                    t = xpool.tile([128, 2 * BLK], MDT,
                                   name=f"xt0_{kt}_{bp}", tag=f"xt{kt}_{bp}")
                    nc.sync.dma_start(
                        t[:], xt_d.ap()[0, kt * 128:(kt + 1) * 128,
                                        bp * 2 * BLK:(bp + 1) * 2 * BLK])
                    xt_sb_all[0][kt, bp] = t

            # ---------------- late statics ----------------
            wo_sb = wpool.tile([HD, D], MDT, name="wo_sb")
            nc.sync.dma_start(wo_sb[:], wo_d.ap()[:, :])
            tri_sb = wpool.tile([128, 128], MDT, name="tri_sb")
            nc.sync.dma_start(tri_sb[:], tri_d.ap()[:, :])

            for b in range(B):
                # ------------- load xT (blocked layout with ones cols) -------------
                if b == 0:
                    xt_sb = xt_sb_all[0]     # DMAs already issued above
                else:
                    xt_sb = {}
                    for bp in range(NCH):
                        for kt in range(KT):
                            t = xpool.tile([128, 2 * BLK], MDT,
                                           name=f"xt{b}_{kt}_{bp}",
                                           tag=f"xt{kt}_{bp}")
                            nc.sync.dma_start(
                                t[:], xt_d.ap()[b, kt * 128:(kt + 1) * 128,
                                                bp * 2 * BLK:(bp + 1) * 2 * BLK])
                            xt_sb[kt, bp] = t

                def xblk(kt, p, lo, hi):
                    """cols [lo:hi) of 258-block p of k-tile kt"""
                    return xt_sb[kt, p // 2][:, (p % 2) * BLK + lo:
                                             (p % 2) * BLK + hi]

                def xchunk(kt, c4):
                    """512 data cols of chunk c4 as 2x256 blocked AP"""
                    v = xt_sb[kt, c4].rearrange("p (a c) -> p a c", c=BLK)
                    return v[:, :, 0:256]

                if STAGE < 12:
                    continue
                # ---- fused per-chunk pipeline: stats + projections ----
                # stats_mt layout per chunk c4: cols [12c4:12c4+4]=mean,
                # [+4:+8]=rstd, [+8:+12]=std (ex2/var during build)
                mean_st = smallp.tile([128, 48], F32, name=f"mst{b}", tag="mst")
                scratch = smallp.tile([128, 128], F32, name=f"scr{b}", tag="scr",
                                      bufs=2)
                mean_row = smallp.tile([1, N], MDT, name=f"mrow{b}", tag="mrow")
                s_row = smallp.tile([1, N], MDT, name=f"srow{b}", tag="srow")
                if with_bias:
                    std_row = smallp.tile([1, N], MDT, name=f"drow{b}", tag="drow")
                s_bcast = bigp.tile([128, N], F32, name=f"sbc{b}", tag="sbc")
                qkv_sb = {}
                for ti, nm in enumerate(("q", "k", "v")):
                    qkv_sb[nm] = bigp.tile([HD, N], MDT, name=f"{nm}T{b}",
                                           tag=f"{nm}T")

                for c4 in range(NCH):
                    cm = mean_st[:, 12 * c4:12 * c4 + 4]
                    cr = mean_st[:, 12 * c4 + 4:12 * c4 + 8]
                    cd = mean_st[:, 12 * c4 + 8:12 * c4 + 12]
                    # -- gram matmuls (PE), extraction queued right after --
                    g_tiles = []
                    for i4 in range(4):             # nsub within chunk
                        p = 2 * c4 + i4 // 2
                        half = i4 % 2
                        g_ps = psB.tile([128, BLK], F32, name=f"g{b}_{c4}_{i4}",
                                        tag=f"pvh{i4 % 2}", bufs=2)
                        for kt in range(KT):
                            nc.tensor.matmul(
                                g_ps[:],
                                xblk(kt, p, half * 128, half * 128 + 128),
                                xblk(kt, p, 0, BLK),
                                start=(kt == 0), stop=(kt == KT - 1))
                        g_tiles.append((g_ps, half, i4))
                    # -- projection main matmuls (PE, independent of stats) --
                    pr_tiles = {}
                    for ti, nm in enumerate(("q", "k", "v")):
                        pr_ps = psA.tile([128, 512], F32, name=f"pr{b}{nm}{c4}",
                                         tag="psA", bufs=4)
                        for kt in range(KT):
                            nc.tensor.matmul(pr_ps[:], w_sb[nm, kt],
                                             xchunk(kt, c4),
                                             start=(kt == 0), stop=False)
                        pr_tiles[nm] = pr_ps
                    if STAGE < 14:
                        continue
                    # -- stats extraction (DVE, overlaps proj matmuls) --
                    for g_ps, half, i4 in g_tiles:
                        nc.vector.scalar_tensor_tensor(
                            out=scratch[:, 0:128],
                            in0=g_ps[:, half * 128:half * 128 + 128],
                            scalar=1.0 / D,
                            in1=ident_sb[:],
                            op0=ALU.mult, op1=ALU.mult,
                            accum_out=cd[:, i4:i4 + 1])
                        nc.vector.tensor_scalar(
                            out=cm[:, i4:i4 + 1],
                            in0=g_ps[:, 256:257], scalar1=1.0 / D, scalar2=None,
                            op0=ALU.mult)
                    if STAGE < 16:
                        continue
                    # -- stats math (DVE/ACT, small) --
                    sq = smallp.tile([128, 4], F32, name=f"sq{b}_{c4}", tag="sq",
                                     bufs=2)
                    nc.vector.tensor_mul(sq[:], cm, cm)
                    nc.vector.scalar_tensor_tensor(
                        out=cd, in0=cd, scalar=EPS, in1=sq[:],
                        op0=ALU.add, op1=ALU.subtract)
                    nc.scalar.activation(cd, cd, AF.Sqrt)
                    nc.vector.reciprocal(cr, cd)
                    if STAGE < 18:
                        continue
                    # -- transpose stats block to rows (PE) --
                    st_ps = psB.tile([12, 128], F32, name=f"stp{b}_{c4}",
                                     tag="pvh0", bufs=2)
                    nc.tensor.transpose(st_ps[:],
                                        mean_st[:, 12 * c4:12 * c4 + 12],
                                        ident_sb[:])
                    st_T = smallp.tile([12, 128], MDT, name=f"stT{b}_{c4}",
                                       tag="stT", bufs=2)
                    nc.vector.tensor_copy(st_T[:], st_ps[:])
                    if STAGE < 20:
                        continue
                    sl = slice(c4 * 512, (c4 + 1) * 512)
                    nc.sync.dma_start(mean_row[0:1, sl], st_T[0:4, :])
                    nc.sync.dma_start(s_row[0:1, sl], st_T[4:8, :])
                    if with_bias:
                        nc.sync.dma_start(std_row[0:1, sl], st_T[8:12, :])
                    if STAGE < 22:
                        continue
                    # -- s broadcast (PE + ACT) --
                    bc_ps = psA.tile([128, 512], F32, name=f"bc{b}_{c4}",
                                     tag="psA", bufs=4)
                    nc.tensor.matmul(bc_ps[:], ones_row, s_row[0:1, sl],
                                     start=True, stop=True)
                    nc.scalar.copy(s_bcast[:, sl], bc_ps[:])
                    if STAGE < 30:
                        continue
                    # -- rank-1 corrections + eviction --
                    for ti, nm in enumerate(("q", "k", "v")):
                        pr_ps = pr_tiles[nm]
                        nc.tensor.matmul(
                            pr_ps[:], aux_sb[0:1, ti * 128:(ti + 1) * 128],
                            mean_row[0:1, sl],
                            start=False, stop=not with_bias)
                        if with_bias:
                            nc.tensor.matmul(
                                pr_ps[:], bias_sb[0:1, ti * 128:(ti + 1) * 128],
                                std_row[0:1, sl],
                                start=False, stop=True)
                        nc.vector.tensor_mul(qkv_sb[nm][:, sl], pr_ps[:],
                                             s_bcast[:, sl])

                if STAGE < 40:
                    continue
                # ------------- v -> natural layout with ones cols -------------
                v_sb = bigp.tile([128, NT * 132], MDT, name=f"vnat{b}", tag="vnat")
                vv = v_sb.rearrange("p (n u c) -> p n u c", u=2, c=66)
                tri16 = tri_sb[:, 0:32].rearrange("p (a c) -> p a c", c=2)
                for u in range(2):
                    nc.scalar.activation(vv[:, :, u, 64:66], tri16, AF.Copy,
                                         bias=1.0, scale=0.0)
                for g in range(NT // 4):
                    vt_ps = psA.tile([128, 512], F32, name=f"vt{b}_{g}",
                                     tag="psA", bufs=4)
                    for j in range(4):
                        nt = 4 * g + j
                        nc.tensor.transpose(
                            vt_ps[:, j * 128:(j + 1) * 128],
                            qkv_sb["v"][:, nt * 128:(nt + 1) * 128].bitcast(F32),
                            ident_sb[:])
                    src = vt_ps.rearrange("p (n u c) -> p n u c", u=2, c=64)
                    dst = vv[:, 4 * g:4 * g + 4, :, 0:64]
                    nc.vector.tensor_copy(dst, src)

                def v_aug(jt, h):
                    return v_sb[:, jt * 132 + h * 66: jt * 132 + (h + 1) * 66]

                if STAGE < 50:
                    continue
                # ------------- attention -------------
                attnhat = bigp.tile([HD, N], MDT, name=f"ah{b}", tag="ah")
                for c4 in range(NCH):
                    pv_ps = [psB.tile([66, 512], F32, name=f"pv{b}{c4}_{h}",
                                      tag=f"pvh{h}", bufs=2) for h in range(2)]
                    njt = 4 * c4 + 4
                    for jt in range(njt):
                        off = 0 if jt < 4 * c4 else (jt - 4 * c4) * 128
                        w = 512 - off
                        ps_sc = []
                        for h in range(2):
                            sc = psA.tile([128, 512], F32, name=f"sc{b}{c4}{jt}{h}",
                                          tag="psA", bufs=4)
                            nc.tensor.matmul(
                                sc[:, 0:w],
                                qkv_sb["k"][h * 64:(h + 1) * 64,
                                            jt * 128:(jt + 1) * 128],
                                qkv_sb["q"][h * 64:(h + 1) * 64,
                                            c4 * 512 + off:(c4 + 1) * 512],
                                start=True, stop=True)
                            ps_sc.append(sc)
                        for h in range(2):
                            p_sb = ppool.tile([128, 512], MDT,
                                              name=f"p{b}{c4}{jt}{h}", tag="p",
                                              bufs=6)
                            nc.scalar.activation(p_sb[:, 0:w], ps_sc[h][:, 0:w],
                                                 AF.Exp)
                            if off > 0 or jt == 4 * c4:
                                # diagonal block: mask first 128 cols (keep j<=i)
                                nc.gpsimd.tensor_mul(p_sb[:, 0:128],
                                                     p_sb[:, 0:128], tri_sb[:])
                            nc.tensor.matmul(pv_ps[h][:, off:512], v_aug(jt, h),
                                             p_sb[:, 0:w],
                                             start=(jt == 0),
                                             stop=(jt == njt - 1))
                    # normalize: attnhat[64h:64h+64, chunk] = attn / denom
                    for h in range(2):
                        rd_sb = smallp.tile([1, 512], MDT, name=f"rd{b}{c4}{h}",
                                            tag="rd", bufs=2)
                        with nc.allow_low_precision(reason="f32r denominators"):
                            nc.vector.reciprocal(rd_sb[:], pv_ps[h][64:65, :])
                        rb_ps = psA.tile([64, 512], F32, name=f"rb{b}{c4}{h}",
                                         tag="psA", bufs=4)
                        nc.tensor.matmul(rb_ps[:], ones_row[0:1, 0:64], rd_sb[:],
                                         start=True, stop=True)
                        rb_sb = smallp.tile([64, 512], F32, name=f"rbs{b}{c4}{h}",
                                            tag="rbs", bufs=2)
                        nc.vector.tensor_copy(rb_sb[:], rb_ps[:])
                        nc.vector.tensor_mul(
                            attnhat[h * 64:(h + 1) * 64,
                                    c4 * 512:(c4 + 1) * 512],
                            pv_ps[h][0:64, :], rb_sb[:])
                    if STAGE < 60:
                        continue
                    # -- out projection, one chunk behind (c4-1) to spread
                    #    psum pressure; final chunk handled after the loop --
                    oc_list = [c4 - 1] if c4 > 0 else []
                    if c4 == NCH - 1:
                        oc_list.append(c4)
                    for oc in oc_list:
                      for it in range(4 * oc, 4 * oc + 4):
                          y_sb = smallp.tile([128, D], F32, name=f"y{b}_{it}",
                                             tag="ysb", bufs=3)
                          for e in range(2):
                              y_ps = psA.tile([128, 512], F32, name=f"yp{b}{it}{e}",
                                              tag="psA", bufs=4)
                              nc.tensor.matmul(y_ps[:],
                                               attnhat[:, it * 128:(it + 1) * 128],
                                               wo_sb[:, e * 512:(e + 1) * 512],
                                               start=True, stop=True)
                              if (it + e) % 2 == 0:
                                  nc.scalar.copy(y_sb[:, e * 512:(e + 1) * 512],
                                                 y_ps[:])
                              else:
                                  nc.vector.tensor_copy(
                                      y_sb[:, e * 512:(e + 1) * 512], y_ps[:])
                          nc.sync.dma_start(
                              y_d.ap()[b, it * 128:(it + 1) * 128, :], y_sb[:])



    nc.compile()
    return nc


_PROG_CACHE = {}


def _get_program(with_bias):
    key = (with_bias, USE_F32R, STAGE)
    if key not in _PROG_CACHE:
        _PROG_CACHE[key] = _build_program(with_bias)
    return _PROG_CACHE[key]


def kernel(x, ln_g, ln_b, lnc_g, lnc_b, Wq, Wkv, Wo):
    global LAST_RESULTS
    x = np.ascontiguousarray(np.asarray(x, dtype=np.float32))
    ln_g = np.asarray(ln_g, np.float32); ln_b = np.asarray(ln_b, np.float32)
    lnc_g = np.asarray(lnc_g, np.float32); lnc_b = np.asarray(lnc_b, np.float32)
    Wq = np.asarray(Wq, np.float32); Wkv = np.asarray(Wkv, np.float32)
    Wo = np.asarray(Wo, np.float32)
    scale = DH ** -0.5

    with_bias = bool(np.any(ln_b) or np.any(lnc_b))
    nc = _get_program(with_bias)

    # xT packed with ones cols: [B, D, 8*257]
    xt = np.empty((B, D, 2 * NCH * BLK), np.float32)
    xTt = np.transpose(x, (0, 2, 1))                     # [B, D, N]
    v = xt.reshape(B, D, 2 * NCH, BLK)
    v[:, :, :, 0:256] = xTt.reshape(B, D, 2 * NCH, 256)
    v[:, :, :, 256:258] = 1.0

    tri = np.triu(np.ones((128, 128), np.float32))       # keep col >= row
    ident = np.eye(128, dtype=np.float32)

    in_maps = []
    for c in range(NCORES):
        cs = slice(c * HD, (c + 1) * HD)
        Wq_eff = np.ascontiguousarray(ln_g[:, None] * Wq[:, cs] * scale)
        Wk_eff = np.ascontiguousarray(lnc_g[:, None] * Wkv[:, :H * DH][:, cs])
        Wv_eff = np.ascontiguousarray(lnc_g[:, None] * Wkv[:, H * DH:][:, cs])
        aux = np.zeros((1, 512), np.float32)
        aux[0, 0:128] = -Wq_eff.sum(0)
        aux[0, 128:256] = -Wk_eff.sum(0)
        aux[0, 256:384] = -Wv_eff.sum(0)
        aux[0, 384:512] = 1.0
        m = {
            "xt": xt,
            "wqkv": np.ascontiguousarray(np.concatenate([Wq_eff, Wk_eff, Wv_eff], axis=1)),
            "wo": np.ascontiguousarray(Wo[cs, :]),
            "aux": aux, "tri": tri, "ident": ident,
        }
        if with_bias:
            br = np.zeros((1, 384), np.float32)
            br[0, 0:128] = ln_b @ Wq[:, cs] * scale
            br[0, 128:256] = lnc_b @ Wkv[:, :H * DH][:, cs]
            br[0, 256:384] = lnc_b @ Wkv[:, H * DH:][:, cs]
            m["biasr"] = br
        in_maps.append(m)

    res = run_bass_kernel_spmd(nc, in_maps, core_ids=list(range(NCORES)),
                               trace=TRACE, **TRACE_KWARGS)
    LAST_RESULTS = res
    y = res.results[0]["y"].astype(np.float32)
    for c in range(1, NCORES):
        y += res.results[c]["y"]
    return y

